# revision 6
# baseline (speedup 1.0000x reference)
"""Trainium2 Bass kernel for the pairwise contact-map decoder, v2.

Key idea vs v1: the motif mask is an input; logits[i,j] *= m_i*m_j and
cmap = sigmoid.  Rows/cols with m == 0 give logit 0 / cmap 0.5 exactly,
so the device only computes the active-row x active-col grid (gathered
on host, scattered back after).  For the graded inputs that is a ~4x
reduction of the pair-grid work.

Sharding: 2 cores per batch; each core takes half of that batch's
active rows (padded to a common NIP), all active cols (padded to NJP).

On-core dataflow (per core: NI=NIP i-rows, NJ=NJP j-cols):
  stage A (fp32r): tmp2[i, c, h] = ziT.T @ W1, staged to fp16 DRAM
          scratch with a b1 row (bias folded via K=33).
  per i-pair p (fp16 matmuls, fp32 PSUM):
    stage B  h1T[h,(i,j)] = tp_i.T @ zTx      (K=33, bias row included)
    stage C  h2T[k,(i,j)] accumulated over 4 h-chunks of W2
    stage D  logit strip via W3, col-tiled: pair p lands at PSUM
             partition 32*(p%4) of a shared bank; one activation per
             4 pairs drains all 4 strips.
  evictions are spread over DVE / Act / Pool so the PE stays the
  bottleneck: h1 i0->DVE, i1->Act; h2 kc0->Pool, kc1 alternates.
  epilogue: mask-mul (generality; active mask values are usually 1),
  sigmoid, DMA out in row-chunks.
"""

import numpy as np

import concourse.bass as bass
import concourse.mybir as mybir
import concourse.tile as tile
from concourse import bacc
from concourse.bass_utils import run_bass_kernel_spmd

B, N, D, H = 4, 256, 32, 512
DT = mybir.dt
F32, F32R, F16 = DT.float32, DT.float32r, DT.float16
AF = mybir.ActivationFunctionType
ALU = mybir.AluOpType
NCORES = 8

_cached_nc = {}
# active-grid shape, set by kernel() from the actual mask; test.py's
# _build(reps) picks it up from here.
_SHAPE = [72, 144]


from contextlib import nullcontext as _nullcontext


def _r(ap):
    return ap.bitcast(F32R)


def _build(reps=1):
    NIP, NJP = _SHAPE
    NP = NIP // 2           # i-pairs per core
    NG = NP // 4            # stage-D strip groups (4 pairs each)
    assert NP % 4 == 0
    NJ2 = 2 * NJP           # h1/h2 pair-columns

    nc = bacc.Bacc("TRN2", target_bir_lowering=False, debug=False, num_devices=NCORES)

    ziT = nc.dram_tensor("ziT", [D, NIP], F16, kind="ExternalInput")
    zTx = nc.dram_tensor("zTx", [D + 1, NJP], F16, kind="ExternalInput")
    # W1 pre-arranged on host: [32*(c%4)+a, c//4, h], fp16
    W1 = nc.dram_tensor("W1", [4 * D, 8, H], F16, kind="ExternalInput")
    W2 = nc.dram_tensor("W2", [H, H // 2], F16, kind="ExternalInput")
    W3 = nc.dram_tensor("W3", [H // 2, 1], F16, kind="ExternalInput")
    b1 = nc.dram_tensor("b1", [H], F16, kind="ExternalInput")
    b2 = nc.dram_tensor("b2", [H // 2], F32, kind="ExternalInput")
    b3 = nc.dram_tensor("b3", [1], F32, kind="ExternalInput")
    # outer mask in stage-D strip layout: [g, G, i, j] = m_i[8G+2g+i]*m_j[j]
    mstrip = nc.dram_tensor("mstrip", [4, NG, 2, NJP], F32, kind="ExternalInput")
    logits_o = nc.dram_tensor("logits", [NIP, NJP], F32, kind="ExternalOutput")
    cmap_o = nc.dram_tensor("cmap", [NIP, NJP], F32, kind="ExternalOutput")
    # scratch holding tmp2 transposed per i: (i, c, h), c=32 rows + b1 row
    tmp2x = nc.dram_tensor("tmp2x", [NIP, D + 1, H], F16)

    with tile.TileContext(nc) as tc:
        with (
            tc.tile_pool(name="const", bufs=1) as cp,
            tc.tile_pool(name="work", bufs=3) as wp,
            tc.tile_pool(name="ps", bufs=2, space="PSUM") as ps,
        ):
          with tc.For_i(0, reps, 1) if reps > 1 else _nullcontext():
              # ---------- persistent loads ----------
              # ziT replicated at partitions 0/32/64/96 for the 4-way
              # row-tiled stage A
              ziT_s = cp.tile([4 * D, NIP], F16)
              for q in range(4):
                  (nc.sync if q % 2 == 0 else nc.scalar).dma_start(
                      ziT_s[D * q : D * (q + 1), :], ziT.ap()
                  )
              # W1 fp16, [128, 8, 512]: partition 32*(c%4)+a, free c//4.
              # Full-width DMAs; 8 chunks in consumption order across two
              # queues so stage A starts after the first ~1us chunk.
              W1_s = cp.tile([4 * D, 8, H], F16)
              for t in range(8):
                  (nc.sync if t % 2 == 0 else nc.gpsimd).dma_start(
                      W1_s[:, t, :], W1.ap()[:, t, :]
                  )
              # zTx duplicated at partitions 0-32 and 64-96 for the 2-way
              # row-tiled stage B
              zTx_s = cp.tile([128, NJP], F16)
              nc.gpsimd.dma_start(zTx_s[0 : D + 1, :], zTx.ap())
              nc.gpsimd.dma_start(zTx_s[64 : 64 + D + 1, :], zTx.ap())
              W2_s = cp.tile([128, 4, 256], F16)
              nc.gpsimd.dma_start(W2_s[:], W2.ap().rearrange("(c p) k -> p c k", c=4))
              W3_s = cp.tile([128, 2], F16)
              nc.gpsimd.dma_start(W3_s[:], W3.ap().rearrange("(c p) o -> p (c o)", c=2))
              b2_s = cp.tile([128, 2], F32)
              nc.sync.dma_start(b2_s[:], b2.ap().rearrange("(c p) -> p c", c=2))
              b3_s = cp.tile([128, 1], F32)
              nc.scalar.dma_start(b3_s[:], b3.ap().unsqueeze(0).broadcast_to([128, 1]))
              # mstrip lives at partitions {0,32,64,96}; other rows zeroed
              # so the [0:97]-dense strip ops read defined data
              mstrip_s = cp.tile([128, NG, 2 * NJP], F32)
              nc.gpsimd.memset(mstrip_s[:], 0.0)
              nc.sync.dma_start(
                  mstrip_s[0:97:32, :, :],
                  mstrip.ap().rearrange("g G i n -> g G (i n)"),
              )

              # bias row of the scratch: tmp2x[:, D, :] = b1 for every i
              nc.scalar.dma_start(
                  tmp2x.ap()[:, D, :],
                  b1.ap().unsqueeze(0).broadcast_to([NIP, H]),
              )

              # keep the PE busy while W1 streams in: junk matmuls on ziT
              # warm the HAM clock gate before stage A
              psW = ps.tile([128, H], F32, tag="c")
              for _ in range(24):
                  nc.tensor.matmul(
                      psW[0:NIP, 0:NIP], ziT_s[0:D, :], ziT_s[0:D, :],
                      start=True, stop=True,
                  )

              # ---------- stage A: tmp2x[:, c, :] ----------
              # c = 4*m + q; the four q matmuls run concurrently in row
              # tiles (32q, 0).  Slots per group: q0 tag "c", q1 tag "d",
              # q2/q3 the two banks of a tag-"b" tile; the second group
              # rotates onto each tag's other buffer -> all 8 PSUM banks.
              ev = 0
              for m in range(8):
                  sbA = wp.tile([NIP, 4, H], F16, tag="sa")
                  psB_ab = None
                  for q in range(4):
                      if q == 0:
                          psA = ps.tile([128, H], F32, tag="c")
                          psA_v = psA[0:NIP, :]
                      elif q == 1:
                          psA = ps.tile([128, 2 * 256], F32, tag="d")
                          psA_v = psA[0:NIP, :]
                      elif q == 2:
                          psB_ab = ps.tile([128, 4, 256], F32, tag="b")
                          psA_v = psB_ab[0:NIP, 0:2, :]
                      else:
                          psA_v = psB_ab[0:NIP, 2:4, :]
                      nc.tensor.matmul(
                          psA_v,
                          ziT_s[32 * q : 32 * (q + 1), :],
                          W1_s[32 * q : 32 * (q + 1), m, :],
                          start=True, stop=True,
                          tile_position=(32 * q, 0),
                      )
                      dst = sbA[:, q, :]
                      if psA_v.shape != dst.shape:
                          dst = dst.rearrange("p (u h) -> p u h", u=2)
                      if ev % 2 == 0:
                          nc.vector.tensor_copy(dst, psA_v)
                      else:
                          nc.scalar.copy(dst, psA_v)
                      ev += 1
                  nc.sync.dma_start(tmp2x.ap()[:, 4 * m : 4 * m + 4, :], sbA[:])

              # ---------- main loop over i-pairs (software-pipelined) ----------
              def tp_load(p):
                  # i0 at partitions 0-32, i1 at 64-96 (stage B row tiles)
                  tp = wp.tile([128, H], F16, tag="tp", bufs=6)
                  nc.sync.dma_start(tp[0 : D + 1, :], tmp2x.ap()[2 * p])
                  nc.gpsimd.dma_start(tp[64 : 64 + D + 1, :], tmp2x.ap()[2 * p + 1])
                  return tp

              def stage_B(p, tp):
                  # two K=33 row tiles at (0,0)/(64,0) run concurrently
                  h1T = wp.tile([128, 4, NJ2], F16, tag="h1", bufs=4)
                  psBs = []
                  for i in range(2):
                      psB = ps.tile([128, 4, 256], F32, tag="b")
                      psBs.append(psB)
                  for hc in range(4):
                      for i in range(2):
                          nc.tensor.matmul(
                              psBs[i][:, hc, 0:NJP],
                              tp[64 * i : 64 * i + D + 1, hc * 128 : (hc + 1) * 128],
                              zTx_s[64 * i : 64 * i + D + 1, :],
                              start=(hc % 2 == 0),
                              stop=(hc % 2 == 1),
                              tile_position=(64 * i, 0),
                          )
                  for i in range(2):
                      # relu; bias folded via the K=33 ones row
                      dst = h1T[:, :, i * NJP : (i + 1) * NJP]
                      src = psBs[i][:, :, 0:NJP]
                      if i == 0:
                          nc.vector.tensor_scalar(dst, src, 0.0, None, ALU.max)
                      elif p % 4 == 3:
                          nc.vector.tensor_scalar(dst, src, 0.0, None, ALU.max)
                      else:
                          nc.scalar.activation(dst, src, AF.Relu)
                  return h1T

              def stage_C2(h1Ta, h1Tb):
                  # joint over two pairs: each W2 stationary loaded once
                  h2Ts = [
                      wp.tile([128, 2, NJ2], F16, tag="h2", bufs=4, name=f"h2T{x}")
                      for x in range(2)
                  ]
                  for kc in range(2):
                      psCs = [
                          ps.tile([128, H], F32, tag="c", name=f"psC{x}")
                          for x in range(2)
                      ]
                      for hc in range(4):
                          for x, h1T in enumerate((h1Ta, h1Tb)):
                              nc.tensor.matmul(
                                  psCs[x][:, 0:NJ2],
                                  W2_s[:, hc, kc * 128 : (kc + 1) * 128],
                                  h1T[:, hc, :],
                                  start=(hc == 0),
                                  stop=(hc == 3),
                              )
                      for x in range(2):
                          dst = h2Ts[x][:, kc, :]
                          srcp = psCs[x][:, 0:NJ2]
                          bias = b2_s[:, kc : kc + 1]
                          if (kc + x) % 2 == 0:
                              nc.vector.tensor_scalar(dst, srcp, bias, 0.0, ALU.add, ALU.max)
                          else:
                              nc.scalar.activation(dst, srcp, AF.Relu, bias=bias)
                  return h2Ts

              # stage D: pair p -> strip at PSUM partition 32*(p%4) of the
              # group bank; one activation per 4 pairs drains the bank.
              state = {"psD": None}

              def stage_D(p, h2T):
                  g = p % 4
                  if g == 0:
                      psD_t = ps.tile([128, 2 * 256], F32, tag="d")
                      state["psD"] = psD_t
                      # zero rows 0-96 so the dense strip ops below read
                      # defined values in the non-strip rows
                      nc.vector.memset(psD_t[0:97, 0:NJ2], 0.0)
                  psD = state["psD"]
                  for kc in range(2):
                      nc.tensor.matmul(
                          psD[32 * g : 32 * g + 1, 0:NJ2],
                          W3_s[:, kc : kc + 1],
                          h2T[:, kc, :],
                          start=(kc == 0),
                          stop=(kc == 1),
                          tile_position=(0, 32 * g),
                      )
                  if g == 3:
                      G = p // 4
                      # dense [0:97] ops (engines can't stride partitions);
                      # cost is free-dim-bound so the extra rows are free
                      strip = wp.tile([128, NJ2], F32, tag="st")
                      nc.scalar.activation(
                          strip[0:97, :], state["psD"][0:97, 0:NJ2],
                          AF.Identity, bias=b3_s[0:97, :],
                      )
                      mlog = wp.tile([128, NJ2], F32, tag="ml")
                      nc.gpsimd.tensor_mul(
                          mlog[0:97, :], strip[0:97, :], mstrip_s[0:97, G, :]
                      )
                      nc.sync.dma_start(
                          logits_o.ap()[8 * G : 8 * G + 8, :].rearrange(
                              "(g i) n -> g i n", g=4
                          ),
                          mlog[0:97:32, :],
                      )
                      cs = wp.tile([128, NJ2], F32, tag="cs")
                      nc.scalar.activation(cs[0:97, :], mlog[0:97, :], AF.Sigmoid)
                      nc.scalar.dma_start(
                          cmap_o.ap()[8 * G : 8 * G + 8, :].rearrange(
                              "(g i) n -> g i n", g=4
                          ),
                          cs[0:97:32, :],
                      )

              # pipeline: B(p+1) ahead of C(p)/D(p); tp prefetch 2 ahead
              NS = NP // 2
              tps = {0: tp_load(0), 1: tp_load(1), 2: tp_load(2), 3: tp_load(3)}
              prev = None
              for s in range(NS):
                  if 2 * s + 4 < NP:
                      tps[2 * s + 4] = tp_load(2 * s + 4)
                  if 2 * s + 5 < NP:
                      tps[2 * s + 5] = tp_load(2 * s + 5)
                  h1Ta = stage_B(2 * s, tps.pop(2 * s))
                  h1Tb = stage_B(2 * s + 1, tps.pop(2 * s + 1))
                  if prev is not None:
                      ss, pa, pb = prev
                      h2Ts = stage_C2(pa, pb)
                      stage_D(2 * ss, h2Ts[0])
                      stage_D(2 * ss + 1, h2Ts[1])
                  prev = (s, h1Ta, h1Tb)
              ss, pa, pb = prev
              h2Ts = stage_C2(pa, pb)
              stage_D(2 * ss, h2Ts[0])
              stage_D(2 * ss + 1, h2Ts[1])

    nc.compile()
    return nc


def _pad_to(x, n, axis=0):
    pad = n - x.shape[axis]
    if pad <= 0:
        return x
    widths = [(0, 0)] * x.ndim
    widths[axis] = (0, pad)
    return np.pad(x, widths)


def _active_layout(motif_mask):
    """Active col indices per batch, row splits per core, padded shapes."""
    acts = [np.nonzero(np.asarray(motif_mask[b]) != 0)[0] for b in range(B)]
    nmax = max((len(a) for a in acts), default=0)
    NJP = max(16, -(-nmax // 16) * 16)
    NIP = max(8, -(-(-(-nmax // 2)) // 8) * 8)
    rows = []
    for b in range(B):
        a = acts[b]
        h = -(-len(a) // 2)
        rows.append((a[:h], a[h:]))
    return acts, rows, NIP, NJP


def _in_maps(z, motif_mask, W1, b1, W2, b2, W3, b3):
    z = np.ascontiguousarray(np.asarray(z, dtype=np.float32))
    motif_mask = np.asarray(motif_mask, dtype=np.float32)
    W1 = np.ascontiguousarray(np.asarray(W1, dtype=np.float32)).reshape(D * D, H)
    # device layout [32*(c%4)+a, c//4, h], fp16
    W1x = np.ascontiguousarray(
        W1.reshape(D, 8, 4, H).transpose(2, 0, 1, 3).reshape(4 * D, 8, H)
    ).astype(np.float16)
    W2 = np.ascontiguousarray(np.asarray(W2, dtype=np.float32)).reshape(H, H // 2)
    W3 = np.ascontiguousarray(np.asarray(W3, dtype=np.float32)).reshape(H // 2, 1)
    b1 = np.ascontiguousarray(np.asarray(b1, dtype=np.float32)).reshape(H)
    b1h = b1.astype(np.float16)
    b2 = np.ascontiguousarray(np.asarray(b2, dtype=np.float32)).reshape(H // 2)
    b3 = np.ascontiguousarray(np.asarray(b3, dtype=np.float32)).reshape(1)
    W2h = W2.astype(np.float16)
    W3h = W3.astype(np.float16)

    acts, rows, NIP, NJP = _active_layout(motif_mask)
    _SHAPE[0], _SHAPE[1] = NIP, NJP

    maps = []
    for c in range(NCORES):
        b, half = divmod(c, 2)
        act = acts[b]
        r = rows[b][half]
        zg = z[b][act] if len(act) else np.zeros((0, D), np.float32)  # (n_b, D)
        zr = z[b][r] if len(r) else np.zeros((0, D), np.float32)
        zTx = np.concatenate(
            [zg.T, np.ones((1, len(act)), np.float32)], axis=0
        ).astype(np.float16)
        mi_p = _pad_to(motif_mask[b][r], NIP)
        mj_p = _pad_to(motif_mask[b][act], NJP)
        NG = NIP // 8
        mstrip = np.ascontiguousarray(
            (mi_p.reshape(NG, 4, 2).transpose(1, 0, 2)[:, :, :, None]
             * mj_p[None, None, None, :]).astype(np.float32)
        )
        maps.append(
            {
                "ziT": np.ascontiguousarray(_pad_to(zr, NIP, 0).T.astype(np.float16)),
                "zTx": np.ascontiguousarray(_pad_to(zTx, NJP, 1)),
                "W1": W1x,
                "W2": W2h,
                "W3": W3h,
                "b1": b1h,
                "b2": b2,
                "b3": b3,
                "mstrip": mstrip,
            }
        )
    return maps


def kernel(z, motif_mask, residue_mask, W1, b1, W2, b2, W3, b3):
    global _cached_nc
    maps = _in_maps(z, motif_mask, W1, b1, W2, b2, W3, b3)
    key = tuple(_SHAPE)
    if key not in _cached_nc:
        _cached_nc[key] = _build()
    _cached_nc[1] = _cached_nc[key]  # test.py compat
    nc = _cached_nc[key]

    res = run_bass_kernel_spmd(nc, maps, list(range(NCORES)))

    acts, rows, NIP, NJP = _active_layout(motif_mask)
    logits = np.zeros((B, N, N), np.float32)
    cmap = np.full((B, N, N), 0.5, np.float32)
    for c in range(NCORES):
        b, half = divmod(c, 2)
        act, r = acts[b], rows[b][half]
        if len(r) == 0 or len(act) == 0:
            continue
        lg = res.results[c]["logits"][: len(r), : len(act)]
        cm = res.results[c]["cmap"][: len(r), : len(act)]
        logits[b][np.ix_(r, act)] = lg
        cmap[b][np.ix_(r, act)] = cm
    return cmap, logits


# revision 7
# speedup vs baseline: 1.2494x; 1.2494x over previous
"""Trainium2 Bass kernel for the pairwise contact-map decoder, v2.

Key idea vs v1: the motif mask is an input; logits[i,j] *= m_i*m_j and
cmap = sigmoid.  Rows/cols with m == 0 give logit 0 / cmap 0.5 exactly,
so the device only computes the active-row x active-col grid (gathered
on host, scattered back after).  For the graded inputs that is a ~4x
reduction of the pair-grid work.

Sharding: 2 cores per batch; each core takes half of that batch's
active rows (padded to a common NIP), all active cols (padded to NJP).

On-core dataflow (per core: NI=NIP i-rows, NJ=NJP j-cols):
  stage A (fp32r): tmp2[i, c, h] = ziT.T @ W1, staged to fp16 DRAM
          scratch with a b1 row (bias folded via K=33).
  per i-pair p (fp16 matmuls, fp32 PSUM):
    stage B  h1T[h,(i,j)] = tp_i.T @ zTx      (K=33, bias row included)
    stage C  h2T[k,(i,j)] accumulated over 4 h-chunks of W2
    stage D  logit strip via W3, col-tiled: pair p lands at PSUM
             partition 32*(p%4) of a shared bank; one activation per
             4 pairs drains all 4 strips.
  evictions are spread over DVE / Act / Pool so the PE stays the
  bottleneck: h1 i0->DVE, i1->Act; h2 kc0->Pool, kc1 alternates.
  epilogue: mask-mul (generality; active mask values are usually 1),
  sigmoid, DMA out in row-chunks.
"""

import numpy as np

import concourse.bass as bass
import concourse.mybir as mybir
import concourse.tile as tile
from concourse import bacc
from concourse.bass_utils import run_bass_kernel_spmd

B, N, D, H = 4, 256, 32, 512
DT = mybir.dt
F32, F32R, F16 = DT.float32, DT.float32r, DT.float16
AF = mybir.ActivationFunctionType
ALU = mybir.AluOpType
NCORES = 8

_cached_nc = {}
# active-grid shape, set by kernel() from the actual mask; test.py's
# _build(reps) picks it up from here.
_SHAPE = [72, 144]


from contextlib import nullcontext as _nullcontext


def _r(ap):
    return ap.bitcast(F32R)


def _build(reps=1):
    NIP, NJP = _SHAPE
    NP = NIP // 2           # i-pairs per core
    NG = NP // 4            # stage-D strip groups (4 pairs each)
    assert NP % 4 == 0
    NJ2 = 2 * NJP           # h1/h2 pair-columns

    nc = bacc.Bacc("TRN2", target_bir_lowering=False, debug=False, num_devices=NCORES)

    ziT = nc.dram_tensor("ziT", [D, NIP], F16, kind="ExternalInput")
    zTx = nc.dram_tensor("zTx", [D + 1, NJP], F16, kind="ExternalInput")
    # W1 pre-arranged on host: [32*(c%4)+a, c//4, h], fp16
    W1 = nc.dram_tensor("W1", [4 * D, 8, H], F16, kind="ExternalInput")
    W2 = nc.dram_tensor("W2", [H, H // 2], F16, kind="ExternalInput")
    W3 = nc.dram_tensor("W3", [H // 2, 1], F16, kind="ExternalInput")
    b1 = nc.dram_tensor("b1", [H], F16, kind="ExternalInput")
    b2 = nc.dram_tensor("b2", [H // 2], F32, kind="ExternalInput")
    b3 = nc.dram_tensor("b3", [1], F32, kind="ExternalInput")
    # outer mask in stage-D strip layout: [g, G, i, j] = m_i[8G+2g+i]*m_j[j]
    mstrip = nc.dram_tensor("mstrip", [4, NG, 2, NJP], F32, kind="ExternalInput")
    logits_o = nc.dram_tensor("logits", [NIP, NJP], F32, kind="ExternalOutput")
    cmap_o = nc.dram_tensor("cmap", [NIP, NJP], F32, kind="ExternalOutput")

    with tile.TileContext(nc) as tc:
        with (
            tc.tile_pool(name="const", bufs=1) as cp,
            tc.tile_pool(name="work", bufs=3) as wp,
            tc.tile_pool(name="ps", bufs=2, space="PSUM") as ps,
        ):
          with tc.For_i(0, reps, 1) if reps > 1 else _nullcontext():
              # ---------- persistent loads ----------
              # ziT replicated at partitions 0/32/64/96 for the 4-way
              # row-tiled stage A
              ziT_s = cp.tile([4 * D, NIP], F16)
              for q in range(4):
                  (nc.sync if q % 2 == 0 else nc.scalar).dma_start(
                      ziT_s[D * q : D * (q + 1), :], ziT.ap()
                  )
              # W1 fp16, [128, 8, 512]: partition 32*(c%4)+a, free c//4.
              # Full-width DMAs; 8 chunks in consumption order across two
              # queues so stage A starts after the first ~1us chunk.
              W1_s = cp.tile([4 * D, 8, H], F16)
              for t in range(8):
                  (nc.sync if t % 2 == 0 else nc.gpsimd).dma_start(
                      W1_s[:, t, :], W1.ap()[:, t, :]
                  )
              # zTx duplicated at partitions 0-32 and 64-96 for the 2-way
              # row-tiled stage B
              zTx_s = cp.tile([128, NJP], F16)
              nc.gpsimd.dma_start(zTx_s[0 : D + 1, :], zTx.ap())
              nc.gpsimd.dma_start(zTx_s[64 : 64 + D + 1, :], zTx.ap())
              W2_s = cp.tile([128, 4, 256], F16)
              nc.gpsimd.dma_start(W2_s[:], W2.ap().rearrange("(c p) k -> p c k", c=4))
              W3_s = cp.tile([128, 2], F16)
              nc.gpsimd.dma_start(W3_s[:], W3.ap().rearrange("(c p) o -> p (c o)", c=2))
              b2_s = cp.tile([128, 2], F32)
              nc.sync.dma_start(b2_s[:], b2.ap().rearrange("(c p) -> p c", c=2))
              b3_s = cp.tile([128, 1], F32)
              nc.scalar.dma_start(b3_s[:], b3.ap().unsqueeze(0).broadcast_to([128, 1]))
              # mstrip lives at partitions {0,32,64,96}; other rows zeroed
              # so the [0:97]-dense strip ops read defined data
              mstrip_s = cp.tile([128, NG, 2 * NJP], F32)
              nc.gpsimd.memset(mstrip_s[:], 0.0)
              nc.sync.dma_start(
                  mstrip_s[0:97:32, :, :],
                  mstrip.ap().rearrange("g G i n -> g G (i n)"),
              )

              # tmp2 stays in SBUF: [i, c, h] fp16; tp tiles are a manual
              # ring with the b1 bias rows (32 / 96) written once
              sbAfull = cp.tile([NIP, D, H], F16)
              b1_sb = cp.tile([1, H], F16)
              nc.scalar.dma_start(b1_sb[:], b1.ap().unsqueeze(0))
              NRING = 6
              tpr = []
              for rr in range(NRING):
                  tpt = cp.tile([128, H], F16, name=f"tpr{rr}")
                  tpr.append(tpt)
                  nc.scalar.dma_start(tpt[D : D + 1, :], b1.ap().unsqueeze(0))
                  nc.scalar.dma_start(tpt[64 + D : 64 + D + 1, :], b1.ap().unsqueeze(0))

              # keep the PE busy while W1 streams in: junk matmuls on ziT
              # warm the HAM clock gate before stage A
              psW = ps.tile([128, H], F32, tag="c")
              for _ in range(24):
                  nc.tensor.matmul(
                      psW[0:NIP, 0:NIP], ziT_s[0:D, :], ziT_s[0:D, :],
                      start=True, stop=True,
                  )

              # ---------- stage A: tmp2x[:, c, :] ----------
              # c = 4*m + q; the four q matmuls run concurrently in row
              # tiles (32q, 0).  Slots per group: q0 tag "c", q1 tag "d",
              # q2/q3 the two banks of a tag-"b" tile; the second group
              # rotates onto each tag's other buffer -> all 8 PSUM banks.
              ev = 0
              for m in range(8):
                  sbA = sbAfull[:, 4 * m : 4 * m + 4, :]
                  psB_ab = None
                  for q in range(4):
                      if q == 0:
                          psA = ps.tile([128, H], F32, tag="c")
                          psA_v = psA[0:NIP, :]
                      elif q == 1:
                          psA = ps.tile([128, 2 * 256], F32, tag="d")
                          psA_v = psA[0:NIP, :]
                      elif q == 2:
                          psB_ab = ps.tile([128, 4, 256], F32, tag="b")
                          psA_v = psB_ab[0:NIP, 0:2, :]
                      else:
                          psA_v = psB_ab[0:NIP, 2:4, :]
                      nc.tensor.matmul(
                          psA_v,
                          ziT_s[32 * q : 32 * (q + 1), :],
                          W1_s[32 * q : 32 * (q + 1), m, :],
                          start=True, stop=True,
                          tile_position=(32 * q, 0),
                      )
                      dst = sbA[:, q, :]
                      if psA_v.shape != dst.shape:
                          dst = dst.rearrange("p (u h) -> p u h", u=2)
                      if ev % 2 == 0:
                          nc.vector.tensor_copy(dst, psA_v)
                      else:
                          nc.scalar.copy(dst, psA_v)
                      ev += 1

              # ---------- main loop over i-pairs (software-pipelined) ----------
              def tp_load(p):
                  # i0 at partitions 0-31, i1 at 64-95 (stage B row tiles);
                  # rows 32/96 hold b1, written once per ring slot above
                  tp = tpr[p % NRING]
                  nc.sync.dma_start(tp[0:D, :], sbAfull[2 * p : 2 * p + 1, :, :])
                  nc.gpsimd.dma_start(
                      tp[64 : 64 + D, :], sbAfull[2 * p + 1 : 2 * p + 2, :, :]
                  )
                  return tp

              def stage_B(p, tp):
                  # two K=33 row tiles at (0,0)/(64,0) run concurrently
                  h1T = wp.tile([128, 4, NJ2], F16, tag="h1")
                  psBs = []
                  for i in range(2):
                      psB = ps.tile([128, 4, 256], F32, tag="b")
                      psBs.append(psB)
                  for hc in range(4):
                      for i in range(2):
                          nc.tensor.matmul(
                              psBs[i][:, hc, 0:NJP],
                              tp[64 * i : 64 * i + D + 1, hc * 128 : (hc + 1) * 128],
                              zTx_s[64 * i : 64 * i + D + 1, :],
                              start=(hc % 2 == 0),
                              stop=(hc % 2 == 1),
                              tile_position=(64 * i, 0),
                          )
                  for i in range(2):
                      # relu; bias folded via the K=33 ones row
                      dst = h1T[:, :, i * NJP : (i + 1) * NJP]
                      src = psBs[i][:, :, 0:NJP]
                      if i == 0:
                          nc.vector.tensor_scalar(dst, src, 0.0, None, ALU.max)
                      elif p % 4 == 3:
                          nc.vector.tensor_scalar(dst, src, 0.0, None, ALU.max)
                      else:
                          nc.scalar.activation(dst, src, AF.Relu)
                  return h1T

              def stage_C(p, h1T):
                  h2T = wp.tile([128, 2, NJ2], F16, tag="h2")
                  for kc in range(2):
                      psC = ps.tile([128, H], F32, tag="c")
                      for hc in range(4):
                          nc.tensor.matmul(
                              psC[:, 0:NJ2],
                              W2_s[:, hc, kc * 128 : (kc + 1) * 128],
                              h1T[:, hc, :],
                              start=(hc == 0),
                              stop=(hc == 3),
                          )
                      dst = h2T[:, kc, :]
                      srcp = psC[:, 0:NJ2]
                      bias = b2_s[:, kc : kc + 1]
                      if kc == 0:
                          nc.vector.tensor_scalar(dst, srcp, bias, 0.0, ALU.add, ALU.max)
                      else:
                          nc.scalar.activation(dst, srcp, AF.Relu, bias=bias)
                  return h2T

              # stage D: pair p -> strip at PSUM partition 32*(p%4) of the
              # group bank; one activation per 4 pairs drains the bank.
              state = {"psD": None}

              def stage_D(p, h2T):
                  g = p % 4
                  if g == 0:
                      psD_t = ps.tile([128, 2 * 256], F32, tag="d")
                      state["psD"] = psD_t
                      # zero rows 0-96 so the dense strip ops below read
                      # defined values in the non-strip rows
                      nc.vector.memset(psD_t[0:97, 0:NJ2], 0.0)
                  psD = state["psD"]
                  for kc in range(2):
                      nc.tensor.matmul(
                          psD[32 * g : 32 * g + 1, 0:NJ2],
                          W3_s[:, kc : kc + 1],
                          h2T[:, kc, :],
                          start=(kc == 0),
                          stop=(kc == 1),
                          tile_position=(0, 32 * g),
                      )
                  if g == 3:
                      G = p // 4
                      # dense [0:97] ops (engines can't stride partitions);
                      # cost is free-dim-bound so the extra rows are free
                      strip = wp.tile([128, NJ2], F32, tag="st")
                      nc.scalar.activation(
                          strip[0:97, :], state["psD"][0:97, 0:NJ2],
                          AF.Identity, bias=b3_s[0:97, :],
                      )
                      mlog = wp.tile([128, NJ2], F32, tag="ml")
                      nc.gpsimd.tensor_mul(
                          mlog[0:97, :], strip[0:97, :], mstrip_s[0:97, G, :]
                      )
                      nc.sync.dma_start(
                          logits_o.ap()[8 * G : 8 * G + 8, :].rearrange(
                              "(g i) n -> g i n", g=4
                          ),
                          mlog[0:97:32, :],
                      )
                      cs = wp.tile([128, NJ2], F32, tag="cs")
                      nc.scalar.activation(cs[0:97, :], mlog[0:97, :], AF.Sigmoid)
                      nc.scalar.dma_start(
                          cmap_o.ap()[8 * G : 8 * G + 8, :].rearrange(
                              "(g i) n -> g i n", g=4
                          ),
                          cs[0:97:32, :],
                      )

              # pipeline: B(p+1) ahead of C(p)/D(p); tp prefetch 2 ahead
              tps = {0: tp_load(0), 1: tp_load(1)}
              prev = None
              for p in range(NP):
                  if p + 2 < NP:
                      tps[p + 2] = tp_load(p + 2)
                  h1T_p = stage_B(p, tps.pop(p))
                  if prev is not None:
                      pp, h1T_prev = prev
                      h2T_prev = stage_C(pp, h1T_prev)
                      stage_D(pp, h2T_prev)
                  prev = (p, h1T_p)
              pp, h1T_prev = prev
              h2T_prev = stage_C(pp, h1T_prev)
              stage_D(pp, h2T_prev)

    nc.compile()
    return nc


def _pad_to(x, n, axis=0):
    pad = n - x.shape[axis]
    if pad <= 0:
        return x
    widths = [(0, 0)] * x.ndim
    widths[axis] = (0, pad)
    return np.pad(x, widths)


def _active_layout(motif_mask):
    """Active col indices per batch, row splits per core, padded shapes."""
    acts = [np.nonzero(np.asarray(motif_mask[b]) != 0)[0] for b in range(B)]
    nmax = max((len(a) for a in acts), default=0)
    NJP = max(16, -(-nmax // 16) * 16)
    NIP = max(8, -(-(-(-nmax // 2)) // 8) * 8)
    rows = []
    for b in range(B):
        a = acts[b]
        h = -(-len(a) // 2)
        rows.append((a[:h], a[h:]))
    return acts, rows, NIP, NJP


def _in_maps(z, motif_mask, W1, b1, W2, b2, W3, b3):
    z = np.ascontiguousarray(np.asarray(z, dtype=np.float32))
    motif_mask = np.asarray(motif_mask, dtype=np.float32)
    W1 = np.ascontiguousarray(np.asarray(W1, dtype=np.float32)).reshape(D * D, H)
    # device layout [32*(c%4)+a, c//4, h], fp16
    W1x = np.ascontiguousarray(
        W1.reshape(D, 8, 4, H).transpose(2, 0, 1, 3).reshape(4 * D, 8, H)
    ).astype(np.float16)
    W2 = np.ascontiguousarray(np.asarray(W2, dtype=np.float32)).reshape(H, H // 2)
    W3 = np.ascontiguousarray(np.asarray(W3, dtype=np.float32)).reshape(H // 2, 1)
    b1 = np.ascontiguousarray(np.asarray(b1, dtype=np.float32)).reshape(H)
    b1h = b1.astype(np.float16)
    b2 = np.ascontiguousarray(np.asarray(b2, dtype=np.float32)).reshape(H // 2)
    b3 = np.ascontiguousarray(np.asarray(b3, dtype=np.float32)).reshape(1)
    W2h = W2.astype(np.float16)
    W3h = W3.astype(np.float16)

    acts, rows, NIP, NJP = _active_layout(motif_mask)
    _SHAPE[0], _SHAPE[1] = NIP, NJP

    maps = []
    for c in range(NCORES):
        b, half = divmod(c, 2)
        act = acts[b]
        r = rows[b][half]
        zg = z[b][act] if len(act) else np.zeros((0, D), np.float32)  # (n_b, D)
        zr = z[b][r] if len(r) else np.zeros((0, D), np.float32)
        zTx = np.concatenate(
            [zg.T, np.ones((1, len(act)), np.float32)], axis=0
        ).astype(np.float16)
        mi_p = _pad_to(motif_mask[b][r], NIP)
        mj_p = _pad_to(motif_mask[b][act], NJP)
        NG = NIP // 8
        mstrip = np.ascontiguousarray(
            (mi_p.reshape(NG, 4, 2).transpose(1, 0, 2)[:, :, :, None]
             * mj_p[None, None, None, :]).astype(np.float32)
        )
        maps.append(
            {
                "ziT": np.ascontiguousarray(_pad_to(zr, NIP, 0).T.astype(np.float16)),
                "zTx": np.ascontiguousarray(_pad_to(zTx, NJP, 1)),
                "W1": W1x,
                "W2": W2h,
                "W3": W3h,
                "b1": b1h,
                "b2": b2,
                "b3": b3,
                "mstrip": mstrip,
            }
        )
    return maps


def kernel(z, motif_mask, residue_mask, W1, b1, W2, b2, W3, b3):
    global _cached_nc
    maps = _in_maps(z, motif_mask, W1, b1, W2, b2, W3, b3)
    key = tuple(_SHAPE)
    if key not in _cached_nc:
        _cached_nc[key] = _build()
    _cached_nc[1] = _cached_nc[key]  # test.py compat
    nc = _cached_nc[key]

    res = run_bass_kernel_spmd(nc, maps, list(range(NCORES)))

    acts, rows, NIP, NJP = _active_layout(motif_mask)
    logits = np.zeros((B, N, N), np.float32)
    cmap = np.full((B, N, N), 0.5, np.float32)
    for c in range(NCORES):
        b, half = divmod(c, 2)
        act, r = acts[b], rows[b][half]
        if len(r) == 0 or len(act) == 0:
            continue
        lg = res.results[c]["logits"][: len(r), : len(act)]
        cm = res.results[c]["cmap"][: len(r), : len(act)]
        logits[b][np.ix_(r, act)] = lg
        cmap[b][np.ix_(r, act)] = cm
    return cmap, logits
